# revision 37
# baseline (speedup 1.0000x reference)
"""EncoderBlock kernel for 8 Trainium2 NeuronCores (data-parallel over batch).

Contract: kernel(**inputs) takes the FULL inputs of reference.setup_inputs()
and returns the FULL [16, 1024, 768] float32 output.

Strategy: pure data parallelism — 16 batches / 8 cores = 2 batches per core,
weights replicated, zero collectives.  Per core a fused Bass/Tile program runs
LN1 -> QKV -> attention -> proj -> residual -> LN2 -> fc1/gelu -> fc2 ->
(normalized residual) -> LN3.

Precision: all large GEMMs use fp8e4 (e4m3) operands with
perf_mode=DoubleRow (two 128-deep k-tiles contracted per instruction);
weights are pre-scaled by 512 on the host so their magnitudes sit in fp8's
normal range, and the 1/512 unscale folds into the downstream evacuation
ops (or the exp/gelu activation scale).  LayerNorm stats, softmax
denominators and residual adds stay fp32/bf16.  Host-emulated end-to-end
error: ~1.1e-2 max-rel vs the fp32 reference (gate 2e-2).

Schedule: the two batches are software-pipelined so batch1's LN1/QKV/V
(PE+DVE+Pool) runs under batch0's ACT-bound softmax stretch, and batch0's
proj/MLP runs under batch1's softmax stretch.  qkT/ctxT are double-buffered
across batches to allow the overlap.  LN rsqrt runs as exp(-0.5*ln(var+eps))
and the overlapped-batch gelu as 0.5x(1+tanh(0.851x)) so every ACT function
in the busy stretches shares one activation table with the softmax Exp
(act-table reloads cost 1.3us each and the Tile scheduler freely interleaves
ACT work).  Transposes ride the DMA xbar (batch1) or the idle head-phase PE
(batch0).  Cost-model time: ~398us vs ~742us for the fp32r baseline.
"""

import os
import sys

sys.path.insert(0, "/opt/trn_rl_repo")
# The axon NTFF profiling hook is unavailable in this environment; force
# tracing off so an externally-set BASS_TRACE cannot break execution.
os.environ["BASS_NEVER_TRACE"] = "1"

import numpy as np
import ml_dtypes

import concourse.bass as bass
import concourse.tile as tile
from concourse import mybir
from concourse.vector_clock import ScopedClock, VectorClock
from concourse.bass_utils import run_bass_kernel_spmd

F32 = mybir.dt.float32
BF = mybir.dt.bfloat16
F8 = mybir.dt.float8e4
AF = mybir.ActivationFunctionType
ALU = mybir.AluOpType
DR = mybir.MatmulPerfMode.DoubleRow

B, N, D = 16, 1024, 768
H, DH, HID = 12, 64, 3072
NCORES = 8
BLOC = B // NCORES
EPS = 1e-5
TC_N = N // 128   # 8 token tiles / batch
KC_D = D // 128   # 6 feature chunks
MC_H = HID // 128  # 24 hidden chunks
SW = 512.0        # host weight prescale (folded out after each GEMM)
SC = 32.0         # ctx scale, folded into the V values

E4NP = ml_dtypes.float8_e4m3


# ---------------------------------------------------------------------------
# Workarounds: this walrus build rejects >1 sync-wait command per instruction.
# ---------------------------------------------------------------------------
def _patched_drain_and_barrier(self, tick_clock, wait_clock):
    gc = tick_clock.global_clock
    n = len(gc)
    for i in range(n):
        t = gc[i]
        if t <= 0:
            continue
        vec = [0] * n
        vec[i] = t
        nop = self.nc.sync.nop(nofuse=True)
        wait_clock.add_sem_waits(nop.ins, ScopedClock({None: VectorClock(vec)}))
    self.nc.sync.drain()
    self.nc.all_engine_barrier()
    assert self.sems is not None
    popped = self.nc._tile_sem_poison_stack.pop()
    assert popped is self._sem_poison
    self.nc.clear_and_free_semaphores(list(self.sems.allocated().values()))
    self.nc.all_engine_barrier()


tile.TileContext._drain_and_barrier = _patched_drain_and_barrier


def _split_sync_waits(nc, limit=1):
    """Move excess per-instruction sync waits onto same-engine NoOps."""
    n_split = 0
    for fn in nc.m.functions:
        for bb in fn.blocks:
            out = []
            for ins in bb.instructions:
                si = ins.sync_info
                waits = list(si.on_wait) if (si and si.on_wait) else []
                if len(waits) > limit:
                    excess, keep = waits[:-limit], waits[-limit:]
                    for w in excess:
                        nop = mybir.InstNoOp(
                            name=f"{ins.name}-ws{n_split}",
                            engine=ins.engine,
                            ins=[],
                            outs=[],
                            sync_info=mybir.SyncInfo(on_wait=[w], on_update=[]),
                        )
                        n_split += 1
                        out.append(nop)
                    si.on_wait = keep
                out.append(ins)
            bb.instructions = out
    return n_split


def _layer_norm(nc, misc, x_t, eps_t):
    """(mean, rstd) [128,1] via DVE bn_stats for x_t [128, 768].  The rsqrt
    is exp(-0.5*ln(var+eps)) on ACT: Ln and Exp share an act table with the
    softmax Exp, so LayerNorms cause no act-table swaps."""
    xr = x_t.rearrange("p (s d) -> p s d", d=256)
    lt = misc.tile([128, 24], F32, tag="lnb", bufs=3)
    stats = lt[:, 0:18].rearrange("p (s d) -> p s d", d=6)
    for s in range(3):
        nc.vector.bn_stats(out=stats[:, s, :], in_=xr[:, s, :])
    mv = lt[:, 18:20]
    nc.vector.bn_aggr(out=mv, in_=lt[:, 0:18])
    lv = lt[:, 20:21]
    nc.scalar.activation(out=lv, in_=mv[:, 1:2], func=AF.Ln, bias=eps_t)
    rstd = misc.tile([128, 1], F32, tag="rstd", bufs=3)
    nc.scalar.activation(out=rstd, in_=lv, func=AF.Exp, scale=-0.5)
    return mv[:, 0:1], rstd


def _build_nc(reps=1, skip_gb2=False, skip_gb3=False):
    nc = bass.Bass()

    x_d = nc.dram_tensor("x", [BLOC, N, D], F32, kind="ExternalInput")
    wqk_d = nc.dram_tensor("w_qk", [D, 2 * D], F8, kind="ExternalInput")
    # wv_d carries 8 k-tiles: 6 weight chunks + [bias row, zeros] pair
    wv_d = nc.dram_tensor("w_v", [8 * 128, D], F8, kind="ExternalInput")
    bqk_d = nc.dram_tensor("b_qk", [2 * D], F32, kind="ExternalInput")
    pw_d = nc.dram_tensor("p_w", [D, D], F8, kind="ExternalInput")
    pbr_d = nc.dram_tensor("pbr", [D], F32, kind="ExternalInput")
    w1_d = nc.dram_tensor("w1", [D, HID], F8, kind="ExternalInput")
    b1_d = nc.dram_tensor("b1", [HID], F32, kind="ExternalInput")
    w2_d = nc.dram_tensor("w2", [HID, D], F8, kind="ExternalInput")
    b2_d = nc.dram_tensor("b2", [D], F32, kind="ExternalInput")
    g2_d = nc.dram_tensor("g2", [D], F32, kind="ExternalInput")
    bt2_d = nc.dram_tensor("bt2", [D], F32, kind="ExternalInput")
    g3_d = nc.dram_tensor("g3", [D], F32, kind="ExternalInput")
    bt3_d = nc.dram_tensor("bt3", [D], F32, kind="ExternalInput")
    y_d = nc.dram_tensor("y", [BLOC, N, D], F32, kind="ExternalOutput")

    with tile.TileContext(nc, pool_alloc_mode="queue") as tc:
        misc = tc.alloc_tile_pool(name="misc", bufs=2)
        const = tc.alloc_tile_pool(name="const", bufs=1)

        eps_t = const.tile([128, 1], F32)
        nc.vector.memset(eps_t, EPS)
        bqk_t = const.tile([128, 12], F32)
        nc.sync.dma_start(out=bqk_t, in_=bqk_d.rearrange("(c p) -> p c", p=128))
        b1_t = const.tile([128, MC_H], F32)
        nc.sync.dma_start(out=b1_t, in_=b1_d.rearrange("(c p) -> p c", p=128))
        # gelu-via-tanh biases: 0.851*b1 (tanh arg) and 0.5*b1 (linear part)
        b1a_t = const.tile([128, MC_H], F32)
        nc.gpsimd.tensor_scalar(out=b1a_t, in0=b1_t, scalar1=0.851,
                                scalar2=None, op0=ALU.mult)
        b1h_t = const.tile([128, MC_H], F32)
        nc.gpsimd.tensor_scalar(out=b1h_t, in0=b1_t, scalar1=0.5,
                                scalar2=None, op0=ALU.mult)
        row1 = const.tile([1, 128], BF)
        nc.vector.memset(row1, 1.0)
        identb = const.tile([128, 128], BF)
        from concourse.masks import make_identity
        make_identity(nc, identb)

        def load_bc(dd, nm):
            t = const.tile([128, D], F32, name=nm)
            nc.sync.dma_start(out=t, in_=dd[None, :].partition_broadcast(128))
            return t

        pbr_bc = load_bc(pbr_d, "pbr_bc")
        b2_bc = load_bc(b2_d, "b2_bc")
        if not skip_gb2:
            g2_bc = load_bc(g2_d, "g2_bc")
            bt2_bc = load_bc(bt2_d, "bt2_bc")
        if not skip_gb3:
            g3_bc = load_bc(g3_d, "g3_bc")
            bt3_bc = load_bc(bt3_d, "bt3_bc")

        # --- weights, loaded once, fp8, pre-scaled by SW on the host ---
        wqk = const.tile([128, KC_D, 2 * D], F8)
        nc.sync.dma_start(out=wqk, in_=wqk_d.rearrange("(c p) n -> p c n", p=128))
        wv = const.tile([128, 8, D], F8)
        nc.sync.dma_start(out=wv, in_=wv_d.rearrange("(c p) n -> p c n", p=128))
        pw = const.tile([128, KC_D, D], F8)
        w1t = const.tile([128, KC_D, HID], F8)
        w2t = const.tile([128, MC_H, D], F8)

        def load_big_weights():
            nc.sync.dma_start(out=pw, in_=pw_d.rearrange("(c p) n -> p c n", p=128))
            nc.sync.dma_start(out=w1t, in_=w1_d.rearrange("(c p) n -> p c n", p=128))
            nc.sync.dma_start(out=w2t, in_=w2_d.rearrange("(c p) n -> p c n", p=128))

        # --- persistent activation tiles ---
        # xnT has 8 k-tiles: 6 data + [ones-on-partition-0, zeros] for the
        # V-projection bias fold.
        xnT = const.tile([128, 8, N], F8)
        nc.gpsimd.memset(xnT[:, 6:8, :], 0.0)
        nc.gpsimd.memset(xnT[0:1, 6, :], 1.0)
        qkT2 = [const.tile([128, 12, N], F8, name=f"qkT{i}") for i in range(2)]
        ctxT2 = [const.tile([128, KC_D, N], F8, name=f"ctxT{i}") for i in range(2)]
        ffinT = const.tile([128, KC_D, N], F8)
        ffhT = const.tile([128, MC_H, N], F8)
        ffin_r = const.tile([128, TC_N, D], BF)
        # V1: per (token-chunk, head): even head -> V cols 0:64, ones col 64;
        # odd head -> ones col 0, V cols 64:128.  The softmax denominator
        # rides along the PV matmul on the aligned spare partition.
        V1 = const.tile([128, TC_N, H, 128], F8)
        nc.gpsimd.memset(V1[:, :, 0::2, 65:128], 0.0)
        nc.gpsimd.memset(V1[:, :, 1::2, 1:64], 0.0)
        nc.gpsimd.memset(V1[:, :, 0::2, 64:65], 1.0)
        nc.gpsimd.memset(V1[:, :, 1::2, 0:1], 1.0)

        # ------------------------------------------------------------------
        # Per-phase block emitters
        # ------------------------------------------------------------------
        def blk_A(b, tcn, psA=None):
            """LN1 for one token chunk -> xnT (fp8, feature-major).  Batch 0
            transposes on the idle PE (head), batch 1 via the DMA xbar (the
            s1 stretch has DVE/PE busy but idle DMA)."""
            x_t = misc.tile([128, D], F32, tag="x_in", bufs=2)
            nc.sync.dma_start(out=x_t, in_=x_d[b, tcn * 128:(tcn + 1) * 128, :])
            mean, rstd = _layer_norm(nc, misc, x_t, eps_t)
            xn_bf = misc.tile([128, D], BF, tag="xn_bf", bufs=2)
            nc.gpsimd.tensor_scalar(out=xn_bf, in0=x_t, scalar1=mean,
                                    scalar2=rstd, op0=ALU.subtract,
                                    op1=ALU.mult)
            if psA is not None:
                for kc in range(KC_D):
                    pt = psA.tile([128, 128], BF, tag="tp")
                    nc.tensor.transpose(pt, xn_bf[:, kc * 128:(kc + 1) * 128],
                                        identb)
                    nc.vector.tensor_copy(
                        out=xnT[:, kc, tcn * 128:(tcn + 1) * 128], in_=pt)
            else:
                tsc = misc.tile([128, KC_D, 128], BF, tag="tsc", bufs=3)
                nc.sync.dma_start_transpose(out=tsc, in_=xn_bf)
                nc.gpsimd.tensor_copy(
                    out=xnT[:, 0:KC_D, tcn * 128:(tcn + 1) * 128], in_=tsc)

        def blk_B(b, fc, nh, psB):
            """One q/k feature chunk for one token half."""
            ps = psB.tile([128, 512], F32, tag="qk")
            for k2 in range(3):
                nc.tensor.matmul(
                    ps,
                    wqk[:, 2 * k2:2 * k2 + 2, fc * 128:(fc + 1) * 128],
                    xnT[:, 2 * k2:2 * k2 + 2, nh * 512:(nh + 1) * 512],
                    start=(k2 == 0), stop=(k2 == 2), perf_mode=DR)
            out = qkT2[b % 2][:, fc, nh * 512:(nh + 1) * 512]
            if b % 2 == 0:
                nc.scalar.activation(out=out, in_=ps, func=AF.Identity,
                                     bias=bqk_t[:, fc:fc + 1], scale=1.0 / SW)
            else:
                nc.vector.tensor_scalar(out=out, in0=ps, scalar1=1.0 / SW,
                                        scalar2=bqk_t[:, fc:fc + 1],
                                        op0=ALU.mult, op1=ALU.add)

        def blk_C(b, tcn, psC):
            """V projection for one token chunk -> packed V1 (x SC)."""
            ps = psC.tile([128, D], F32, tag="v")
            lhs = xnT[:, :, tcn * 128:(tcn + 1) * 128]
            for k2 in range(4):
                nc.tensor.matmul(ps[:, 0:512],
                                 lhs[:, 2 * k2:2 * k2 + 2, :],
                                 wv[:, 2 * k2:2 * k2 + 2, 0:512],
                                 start=(k2 == 0), stop=(k2 == 3),
                                 perf_mode=DR)
                nc.tensor.matmul(ps[:, 512:768],
                                 lhs[:, 2 * k2:2 * k2 + 2, :],
                                 wv[:, 2 * k2:2 * k2 + 2, 512:768],
                                 start=(k2 == 0), stop=(k2 == 3),
                                 perf_mode=DR)
            pv3 = ps.rearrange("p (h d) -> p h d", d=DH)
            s = SC / SW
            if b % 2 == 0:
                nc.scalar.activation(out=V1[:, tcn, 0::2, 0:64],
                                     in_=pv3[:, 0::2, :], func=AF.Identity,
                                     scale=s)
                nc.scalar.activation(out=V1[:, tcn, 1::2, 64:128],
                                     in_=pv3[:, 1::2, :], func=AF.Identity,
                                     scale=s)
            else:
                nc.vector.tensor_scalar(out=V1[:, tcn, 0::2, 0:64],
                                        in0=pv3[:, 0::2, :], scalar1=s,
                                        scalar2=None, op0=ALU.mult)
                nc.vector.tensor_scalar(out=V1[:, tcn, 1::2, 64:128],
                                        in0=pv3[:, 1::2, :], scalar1=s,
                                        scalar2=None, op0=ALU.mult)

        def blk_D(b, nh, h, psS, psX, pPT):
            """Attention for one (query-half, head)."""
            qkT = qkT2[b % 2]
            qh = (h % 2) * 64
            dh = 64 if h % 2 == 0 else 0  # denominator partition
            fq, fk = h // 2, 6 + h // 2
            PT = pPT.tile([128, TC_N, 512], F8, tag="PT", bufs=2)
            for mc2 in range(TC_N // 2):
                sp = psS.tile([128, 1024], F32, tag="sc")
                for mi in range(2):
                    mc = mc2 * 2 + mi
                    nc.tensor.matmul(
                        sp[:, mi * 512:(mi + 1) * 512],
                        qkT[qh:qh + 64, fk, mc * 128:(mc + 1) * 128],
                        qkT[qh:qh + 64, fq, nh * 512:(nh + 1) * 512],
                        start=True, stop=True)
                nc.scalar.activation(
                    out=PT[:, mc2 * 2:mc2 * 2 + 2, :], in_=sp,
                    func=AF.Exp, scale=0.125)
            cp = psX.tile([128, 512], F32, tag="cp")
            for m2 in range(TC_N // 2):
                nc.tensor.matmul(cp, V1[:, 2 * m2:2 * m2 + 2, h, :],
                                 PT[:, 2 * m2:2 * m2 + 2, :],
                                 start=(m2 == 0), stop=(m2 == 3),
                                 perf_mode=DR)
            rd = misc.tile([1, 512], BF, tag="rd", name="rd")
            with nc.allow_low_precision(reason="bf16 recip"):
                nc.vector.reciprocal(out=rd, in_=cp[dh:dh + 1, :])
            bb = psX.tile([128, 512], F32, tag="bb")
            nc.tensor.matmul(bb, row1, rd, start=True, stop=True)
            bsb = misc.tile([128, 512], BF, tag="bbs", bufs=2)
            nc.vector.tensor_copy(out=bsb[qh:qh + 64, :], in_=bb[qh:qh + 64, :])
            nc.vector.tensor_tensor(
                out=ctxT2[b % 2][qh:qh + 64, fq, nh * 512:(nh + 1) * 512],
                in0=cp[qh:qh + 64, :], in1=bsb[qh:qh + 64, :],
                op=ALU.mult)

        def blk_E(b, tcn, psE):
            """proj + residual -> x2 -> LN2 -> ffin_r + ffinT."""
            ps = psE.tile([128, D], F32, tag="pj")
            x_t = misc.tile([128, D], F32, tag="x_in", bufs=2)
            nc.sync.dma_start(out=x_t, in_=x_d[b, tcn * 128:(tcn + 1) * 128, :])
            nc.gpsimd.tensor_tensor(out=x_t, in0=x_t, in1=pbr_bc, op=ALU.add)
            lhs = ctxT2[b % 2][:, :, tcn * 128:(tcn + 1) * 128]
            for k2 in range(3):
                nc.tensor.matmul(ps[:, 0:512],
                                 lhs[:, 2 * k2:2 * k2 + 2, :],
                                 pw[:, 2 * k2:2 * k2 + 2, 0:512],
                                 start=(k2 == 0), stop=(k2 == 2),
                                 perf_mode=DR)
                nc.tensor.matmul(ps[:, 512:768],
                                 lhs[:, 2 * k2:2 * k2 + 2, :],
                                 pw[:, 2 * k2:2 * k2 + 2, 512:768],
                                 start=(k2 == 0), stop=(k2 == 2),
                                 perf_mode=DR)
            x2 = misc.tile([128, D], F32, tag="xwork", bufs=2)
            nc.vector.scalar_tensor_tensor(
                out=x2, in0=ps, scalar=1.0 / (SW * SC), in1=x_t,
                op0=ALU.mult, op1=ALU.add)
            mean, rstd = _layer_norm(nc, misc, x2, eps_t)
            fi = ffin_r[:, tcn, :]
            nc.gpsimd.tensor_scalar(out=fi, in0=x2, scalar1=mean,
                                    scalar2=rstd, op0=ALU.subtract,
                                    op1=ALU.mult)
            if not skip_gb2:
                nc.gpsimd.tensor_tensor(out=fi, in0=fi, in1=g2_bc, op=ALU.mult)
                nc.gpsimd.tensor_tensor(out=fi, in0=fi, in1=bt2_bc, op=ALU.add)
            tsc = misc.tile([128, KC_D, 128], BF, tag="tsc", bufs=3)
            nc.sync.dma_start_transpose(out=tsc, in_=fi)
            nc.gpsimd.tensor_copy(
                out=ffinT[:, :, tcn * 128:(tcn + 1) * 128], in_=tsc)

        def blk_F(b, mc, psF, half=None):
            """fc1 + gelu for one hidden chunk (all 1024 tokens or one half)."""
            nhs = range(2) if half is None else [half]
            w = N if half is None else 512
            ps = psF.tile([128, w], F32, tag="f1")
            for k2 in range(3):
                w1s = w1t[:, 2 * k2:2 * k2 + 2, mc * 128:(mc + 1) * 128]
                for j, nh in enumerate(nhs):
                    nc.tensor.matmul(
                        ps[:, j * 512:(j + 1) * 512],
                        w1s,
                        ffinT[:, 2 * k2:2 * k2 + 2, nh * 512:(nh + 1) * 512],
                        start=(k2 == 0), stop=(k2 == 2),
                        perf_mode=DR)
            out = ffhT[:, mc, :] if half is None else \
                ffhT[:, mc, half * 512:(half + 1) * 512]
            if b % 2 == 0:
                # gelu(x) ~ 0.5x(1+tanh(0.851x)); Tanh shares the act table
                # with Exp so interleaving with softmax causes no table swaps.
                th = misc.tile([128, w], BF, tag="tnh", bufs=2)
                nc.scalar.activation(out=th, in_=ps, func=AF.Tanh,
                                     bias=b1a_t[:, mc:mc + 1], scale=0.851 / SW)
                u = misc.tile([128, w], BF, tag="uu", bufs=2)
                if mc % 2 == 0:
                    nc.scalar.activation(out=u, in_=ps, func=AF.Identity,
                                         bias=b1h_t[:, mc:mc + 1],
                                         scale=0.5 / SW)
                else:
                    nc.vector.tensor_scalar(out=u, in0=ps, scalar1=0.5 / SW,
                                            scalar2=b1h_t[:, mc:mc + 1],
                                            op0=ALU.mult, op1=ALU.add)
                nc.vector.scalar_tensor_tensor(out=out, in0=th,
                                               scalar=1.0, in1=u,
                                               op0=ALU.add, op1=ALU.mult)
            else:
                # tail: no softmax exps left, exact gelu costs one table load
                nc.scalar.activation(out=out, in_=ps, func=AF.Gelu,
                                     bias=b1_t[:, mc:mc + 1], scale=1.0 / SW)

        def blk_G(b, ti, psG):
            """fc2 + residual + LN3 -> y."""
            ps = psG.tile([128, D], F32, tag="f2")
            lhs = ffhT[:, :, ti * 128:(ti + 1) * 128]
            for k2 in range(MC_H // 2):
                nc.tensor.matmul(ps[:, 0:512],
                                 lhs[:, 2 * k2:2 * k2 + 2, :],
                                 w2t[:, 2 * k2:2 * k2 + 2, 0:512],
                                 start=(k2 == 0), stop=(k2 == 11),
                                 perf_mode=DR)
                nc.tensor.matmul(ps[:, 512:768],
                                 lhs[:, 2 * k2:2 * k2 + 2, :],
                                 w2t[:, 2 * k2:2 * k2 + 2, 512:768],
                                 start=(k2 == 0), stop=(k2 == 11),
                                 perf_mode=DR)
            x3 = misc.tile([128, D], F32, tag="xwork", bufs=2)
            nc.vector.scalar_tensor_tensor(
                out=x3, in0=ps, scalar=1.0 / SW, in1=ffin_r[:, ti, :],
                op0=ALU.mult, op1=ALU.add)
            nc.gpsimd.tensor_tensor(out=x3, in0=x3, in1=b2_bc, op=ALU.add)
            mean, rstd = _layer_norm(nc, misc, x3, eps_t)
            yt = misc.tile([128, D], F32, tag="xout", bufs=2)
            nc.gpsimd.tensor_scalar(out=yt, in0=x3, scalar1=mean,
                                    scalar2=rstd,
                                    op0=ALU.subtract, op1=ALU.mult)
            if not skip_gb3:
                nc.gpsimd.tensor_tensor(out=yt, in0=yt, in1=g3_bc, op=ALU.mult)
                nc.gpsimd.tensor_tensor(out=yt, in0=yt, in1=bt3_bc, op=ALU.add)
            nc.sync.dma_start(out=y_d[b, ti * 128:(ti + 1) * 128, :], in_=yt)

        # ------------------------------------------------------------------
        # Emission schedule (software pipeline over the BLOC=2 batches).
        # ------------------------------------------------------------------
        def interleave(primary, secondary):
            """Emit primary blocks with secondary blocks spread between."""
            np_, ns = len(primary), len(secondary)
            si = 0
            for i, p in enumerate(primary):
                p()
                want = (i + 1) * ns // np_
                while si < want:
                    secondary[si]()
                    si += 1
            while si < ns:
                secondary[si]()
                si += 1

        b_pair_order = [(fc, nh) for p in range(6)
                        for fc in (p, 6 + p) for nh in range(2)]

        for rep in range(reps):
            r0, r1 = 2 * rep, 2 * rep + 1

            # -- head: A0 with B0/C0 interleaved per token-half --
            psA0 = tc.alloc_tile_pool(name=f"psA{r0}", bufs=2, space="PSUM")
            psB0 = tc.alloc_tile_pool(name=f"psB{r0}", bufs=2, space="PSUM")
            psC0 = tc.alloc_tile_pool(name=f"psC{r0}", bufs=2, space="PSUM")
            for tcn in range(4):
                blk_A(0, tcn, psA0)
            pri = [(lambda t=t: blk_A(0, t, psA0)) for t in range(4, TC_N)]
            pri += [(lambda t=t: blk_C(0, t, psC0)) for t in range(4)]
            sec = [(lambda fc=fc, nh=nh: blk_B(0, fc, nh, psB0))
                   for fc, nh in b_pair_order if nh == 0]
            interleave(pri, sec)
            pri = [(lambda t=t: blk_C(0, t, psC0)) for t in range(4, TC_N)]
            sec = [(lambda fc=fc, nh=nh: blk_B(0, fc, nh, psB0))
                   for fc, nh in b_pair_order if nh == 1]
            interleave(pri, sec)
            psC0.release()
            psB0.release()
            psA0.release()

            # -- stretch 1: D0 (ACT-bound) over A1 B1 --
            psS0 = tc.alloc_tile_pool(name=f"psS{r0}", bufs=2, space="PSUM")
            psX0 = tc.alloc_tile_pool(name=f"psX{r0}", bufs=1, space="PSUM")
            pPT0 = tc.alloc_tile_pool(name=f"pPT{r0}", bufs=2)
            psB1 = tc.alloc_tile_pool(name=f"psB{r1}", bufs=1, space="PSUM")
            sec = [load_big_weights]
            sec += [(lambda t=t: blk_A(1, t)) for t in range(TC_N)]
            sec += [(lambda fc=fc, nh=nh: blk_B(1, fc, nh, psB1))
                    for fc, nh in b_pair_order]
            pri = [(lambda nh=nh, h=h: blk_D(0, nh, h, psS0, psX0, pPT0))
                   for nh in range(2) for h in range(H)]
            interleave(pri, sec)
            psB1.release()
            psX0.release()
            psS0.release()
            pPT0.release()

            # -- stretch 2 psum reserved first so D1 scores/exp overlap C1 --
            psS1 = tc.alloc_tile_pool(name=f"psS{r1}", bufs=2, space="PSUM")
            psX1 = tc.alloc_tile_pool(name=f"psX{r1}", bufs=1, space="PSUM")
            pPT1 = tc.alloc_tile_pool(name=f"pPT{r1}", bufs=2)

            # -- gap: C1 (V projection for batch 1) --
            psC1 = tc.alloc_tile_pool(name=f"psC{r1}", bufs=1, space="PSUM")
            for tcn in range(TC_N):
                blk_C(1, tcn, psC1)
            psC1.release()
            efg_pools = {}

            def efg_open(key, bufs=1):
                efg_pools[key] = tc.alloc_tile_pool(
                    name=f"ps{key}{r0}", bufs=bufs, space="PSUM")

            sec = [lambda: efg_open("E")]
            sec += [(lambda t=t: blk_E(0, t, efg_pools["E"])) for t in range(TC_N)]
            sec += [lambda: efg_pools["E"].release(), lambda: efg_open("F")]

            def f_burst(b, mc0):
                for mc in range(mc0, mc0 + 8):
                    blk_F(b, mc, efg_pools["F"])

            sec += [(lambda m=m: f_burst(0, m)) for m in range(0, MC_H, 8)]
            sec += [lambda: efg_pools["F"].release(), lambda: efg_open("G")]
            sec += [(lambda t=t: blk_G(0, t, efg_pools["G"])) for t in range(TC_N)]
            sec += [lambda: efg_pools["G"].release(), lambda: efg_open("E1", 1)]
            sec += [(lambda t=t: blk_E(1, t, efg_pools["E1"])) for t in range(4)]
            pri = [(lambda nh=nh, h=h: blk_D(1, nh, h, psS1, psX1, pPT1))
                   for nh in range(2) for h in range(H)]
            interleave(pri, sec)
            efg_pools["E1"].release()
            psX1.release()
            psS1.release()
            pPT1.release()

            # -- tail: E1 F1 G1 --
            psE1 = tc.alloc_tile_pool(name=f"psE{r1}b", bufs=3, space="PSUM")
            for tcn in range(4, TC_N):
                blk_E(1, tcn, psE1)
            psE1.release()
            psF1 = tc.alloc_tile_pool(name=f"psF{r1}", bufs=3, space="PSUM")
            for mc in range(MC_H):
                blk_F(1, mc, psF1)
            psF1.release()
            psG1 = tc.alloc_tile_pool(name=f"psG{r1}", bufs=3, space="PSUM")
            for ti in range(TC_N):
                blk_G(1, ti, psG1)
            psG1.release()

        const.release()
        misc.release()

    _split_sync_waits(nc)
    return nc


_NC_CACHE = {}


def _get_nc(reps=1, skip_gb2=False, skip_gb3=False):
    key = f"nc{reps}_{skip_gb2}_{skip_gb3}"
    if key not in _NC_CACHE:
        _NC_CACHE[key] = _build_nc(reps, skip_gb2, skip_gb3)
    return _NC_CACHE[key]


def kernel(x, ln1_g, ln1_b, qkv_w, qkv_b, proj_w, proj_b,
           ln2_g, ln2_b, fc1_w, fc1_b, fc2_w, fc2_b, ln3_g, ln3_b,
           **extra):
    x = np.ascontiguousarray(np.asarray(x, np.float32))
    f = lambda a: np.ascontiguousarray(np.asarray(a, np.float32))
    f8 = lambda a: np.ascontiguousarray(np.asarray(SW * a, E4NP))
    qkv_w, qkv_b = f(qkv_w), f(qkv_b)

    # Fold LN1 gamma/beta into QKV weights/bias (host, fp32).
    w_eff = np.asarray(ln1_g, np.float32)[:, None] * qkv_w
    b_eff = np.asarray(ln1_b, np.float32) @ qkv_w + qkv_b

    # V weights: 6 k-tile chunks of SW*w_v + a bias k-tile pair whose first
    # row is SW*b_v (multiplied on-device by the ones row in xnT k-tile 6).
    wv_pack = np.zeros((8 * 128, D), np.float32)
    wv_pack[:D] = SW * w_eff[:, 2 * D:]
    wv_pack[D] = SW * b_eff[2 * D:]

    common = {
        "w_qk": f8(w_eff[:, :2 * D]),
        "w_v": np.ascontiguousarray(np.asarray(wv_pack, E4NP)),
        "b_qk": f(b_eff[:2 * D]),
        "p_w": f8(proj_w), "pbr": f(proj_b),
        "w1": f8(fc1_w), "b1": f(fc1_b),
        "w2": f8(fc2_w), "b2": f(fc2_b),
        "g2": f(ln2_g), "bt2": f(ln2_b),
        "g3": f(ln3_g), "bt3": f(ln3_b),
    }
    in_maps = [dict(common, x=x[i * BLOC:(i + 1) * BLOC]) for i in range(NCORES)]

    skip_gb2 = bool(np.all(common["g2"] == 1.0) and np.all(common["bt2"] == 0.0))
    skip_gb3 = bool(np.all(common["g3"] == 1.0) and np.all(common["bt3"] == 0.0))
    nc = _get_nc(1, skip_gb2, skip_gb3)
    res = run_bass_kernel_spmd(nc, in_maps, core_ids=list(range(NCORES)))
    _NC_CACHE["last_result"] = res
    return np.concatenate([r["y"] for r in res.results], axis=0)


# revision 38
# speedup vs baseline: 1.0077x; 1.0077x over previous
"""EncoderBlock kernel for 8 Trainium2 NeuronCores (data-parallel over batch).

Contract: kernel(**inputs) takes the FULL inputs of reference.setup_inputs()
and returns the FULL [16, 1024, 768] float32 output.

Strategy: pure data parallelism — 16 batches / 8 cores = 2 batches per core,
weights replicated, zero collectives.  Per core a fused Bass/Tile program runs
LN1 -> QKV -> attention -> proj -> residual -> LN2 -> fc1/gelu -> fc2 ->
(normalized residual) -> LN3.

Precision: all large GEMMs use fp8e4 (e4m3) operands with
perf_mode=DoubleRow (two 128-deep k-tiles contracted per instruction);
weights are pre-scaled by 512 on the host so their magnitudes sit in fp8's
normal range, and the 1/512 unscale folds into the downstream evacuation
ops (or the exp/gelu activation scale).  LayerNorm stats, softmax
denominators and residual adds stay fp32/bf16.  Host-emulated end-to-end
error: ~1.1e-2 max-rel vs the fp32 reference (gate 2e-2).

Schedule: the two batches are software-pipelined so batch1's LN1/QKV/V
(PE+DVE+Pool) runs under batch0's ACT-bound softmax stretch, and batch0's
proj/MLP runs under batch1's softmax stretch.  qkT/ctxT are double-buffered
across batches to allow the overlap.  LN rsqrt runs as exp(-0.5*ln(var+eps))
and the overlapped-batch gelu as 0.5x(1+tanh(0.851x)) so every ACT function
in the busy stretches shares one activation table with the softmax Exp
(act-table reloads cost 1.3us each and the Tile scheduler freely interleaves
ACT work).  Transposes ride the DMA xbar (batch1) or the idle head-phase PE
(batch0).  Cost-model time: ~398us vs ~742us for the fp32r baseline.
"""

import os
import sys

sys.path.insert(0, "/opt/trn_rl_repo")
# The axon NTFF profiling hook is unavailable in this environment; force
# tracing off so an externally-set BASS_TRACE cannot break execution.
os.environ["BASS_NEVER_TRACE"] = "1"

import numpy as np
import ml_dtypes

import concourse.bass as bass
import concourse.tile as tile
from concourse import mybir
from concourse.vector_clock import ScopedClock, VectorClock
from concourse.bass_utils import run_bass_kernel_spmd

F32 = mybir.dt.float32
BF = mybir.dt.bfloat16
F8 = mybir.dt.float8e4
AF = mybir.ActivationFunctionType
ALU = mybir.AluOpType
DR = mybir.MatmulPerfMode.DoubleRow

B, N, D = 16, 1024, 768
H, DH, HID = 12, 64, 3072
NCORES = 8
BLOC = B // NCORES
EPS = 1e-5
TC_N = N // 128   # 8 token tiles / batch
KC_D = D // 128   # 6 feature chunks
MC_H = HID // 128  # 24 hidden chunks
SW = 512.0        # host weight prescale (folded out after each GEMM)
SC = 32.0         # ctx scale, folded into the V values

E4NP = ml_dtypes.float8_e4m3


# ---------------------------------------------------------------------------
# Workarounds: this walrus build rejects >1 sync-wait command per instruction.
# ---------------------------------------------------------------------------
def _patched_drain_and_barrier(self, tick_clock, wait_clock):
    gc = tick_clock.global_clock
    n = len(gc)
    for i in range(n):
        t = gc[i]
        if t <= 0:
            continue
        vec = [0] * n
        vec[i] = t
        nop = self.nc.sync.nop(nofuse=True)
        wait_clock.add_sem_waits(nop.ins, ScopedClock({None: VectorClock(vec)}))
    self.nc.sync.drain()
    self.nc.all_engine_barrier()
    assert self.sems is not None
    popped = self.nc._tile_sem_poison_stack.pop()
    assert popped is self._sem_poison
    self.nc.clear_and_free_semaphores(list(self.sems.allocated().values()))
    self.nc.all_engine_barrier()


tile.TileContext._drain_and_barrier = _patched_drain_and_barrier


def _split_sync_waits(nc, limit=1):
    """Move excess per-instruction sync waits onto same-engine NoOps."""
    n_split = 0
    for fn in nc.m.functions:
        for bb in fn.blocks:
            out = []
            for ins in bb.instructions:
                si = ins.sync_info
                waits = list(si.on_wait) if (si and si.on_wait) else []
                if len(waits) > limit:
                    excess, keep = waits[:-limit], waits[-limit:]
                    for w in excess:
                        nop = mybir.InstNoOp(
                            name=f"{ins.name}-ws{n_split}",
                            engine=ins.engine,
                            ins=[],
                            outs=[],
                            sync_info=mybir.SyncInfo(on_wait=[w], on_update=[]),
                        )
                        n_split += 1
                        out.append(nop)
                    si.on_wait = keep
                out.append(ins)
            bb.instructions = out
    return n_split


def _layer_norm(nc, misc, x_t, eps_t):
    """(mean, rstd) [128,1] via DVE bn_stats for x_t [128, 768].  The rsqrt
    is exp(-0.5*ln(var+eps)) on ACT: Ln and Exp share an act table with the
    softmax Exp, so LayerNorms cause no act-table swaps."""
    xr = x_t.rearrange("p (s d) -> p s d", d=256)
    lt = misc.tile([128, 24], F32, tag="lnb", bufs=3)
    stats = lt[:, 0:18].rearrange("p (s d) -> p s d", d=6)
    for s in range(3):
        nc.vector.bn_stats(out=stats[:, s, :], in_=xr[:, s, :])
    mv = lt[:, 18:20]
    nc.vector.bn_aggr(out=mv, in_=lt[:, 0:18])
    lv = lt[:, 20:21]
    nc.scalar.activation(out=lv, in_=mv[:, 1:2], func=AF.Ln, bias=eps_t)
    rstd = misc.tile([128, 1], F32, tag="rstd", bufs=3)
    nc.scalar.activation(out=rstd, in_=lv, func=AF.Exp, scale=-0.5)
    return mv[:, 0:1], rstd


def _build_nc(reps=1, skip_gb2=False, skip_gb3=False):
    nc = bass.Bass()

    x_d = nc.dram_tensor("x", [BLOC, N, D], F32, kind="ExternalInput")
    wqk_d = nc.dram_tensor("w_qk", [D, 2 * D], F8, kind="ExternalInput")
    # wv_d carries 8 k-tiles: 6 weight chunks + [bias row, zeros] pair
    wv_d = nc.dram_tensor("w_v", [8 * 128, D], F8, kind="ExternalInput")
    bqk_d = nc.dram_tensor("b_qk", [2 * D], F32, kind="ExternalInput")
    pw_d = nc.dram_tensor("p_w", [D, D], F8, kind="ExternalInput")
    pbr_d = nc.dram_tensor("pbr", [D], F32, kind="ExternalInput")
    w1_d = nc.dram_tensor("w1", [D, HID], F8, kind="ExternalInput")
    b1_d = nc.dram_tensor("b1", [HID], F32, kind="ExternalInput")
    w2_d = nc.dram_tensor("w2", [HID, D], F8, kind="ExternalInput")
    b2_d = nc.dram_tensor("b2", [D], F32, kind="ExternalInput")
    g2_d = nc.dram_tensor("g2", [D], F32, kind="ExternalInput")
    bt2_d = nc.dram_tensor("bt2", [D], F32, kind="ExternalInput")
    g3_d = nc.dram_tensor("g3", [D], F32, kind="ExternalInput")
    bt3_d = nc.dram_tensor("bt3", [D], F32, kind="ExternalInput")
    y_d = nc.dram_tensor("y", [BLOC, N, D], F32, kind="ExternalOutput")

    with tile.TileContext(nc, pool_alloc_mode="queue") as tc:
        misc = tc.alloc_tile_pool(name="misc", bufs=2)
        const = tc.alloc_tile_pool(name="const", bufs=1)

        eps_t = const.tile([128, 1], F32)
        nc.vector.memset(eps_t, EPS)
        bqk_t = const.tile([128, 12], F32)
        nc.sync.dma_start(out=bqk_t, in_=bqk_d.rearrange("(c p) -> p c", p=128))
        b1_t = const.tile([128, MC_H], F32)
        nc.sync.dma_start(out=b1_t, in_=b1_d.rearrange("(c p) -> p c", p=128))
        # gelu-via-tanh biases: 0.851*b1 (tanh arg) and 0.5*b1 (linear part)
        b1a_t = const.tile([128, MC_H], F32)
        nc.gpsimd.tensor_scalar(out=b1a_t, in0=b1_t, scalar1=0.851,
                                scalar2=None, op0=ALU.mult)
        b1h_t = const.tile([128, MC_H], F32)
        nc.gpsimd.tensor_scalar(out=b1h_t, in0=b1_t, scalar1=0.5,
                                scalar2=None, op0=ALU.mult)
        row1 = const.tile([1, 128], BF)
        nc.vector.memset(row1, 1.0)
        identb = const.tile([128, 128], BF)
        from concourse.masks import make_identity
        make_identity(nc, identb)

        def load_bc(dd, nm):
            t = const.tile([128, D], F32, name=nm)
            nc.sync.dma_start(out=t, in_=dd[None, :].partition_broadcast(128))
            return t

        pbr_bc = load_bc(pbr_d, "pbr_bc")
        b2_bc = load_bc(b2_d, "b2_bc")
        if not skip_gb2:
            g2_bc = load_bc(g2_d, "g2_bc")
            bt2_bc = load_bc(bt2_d, "bt2_bc")
        if not skip_gb3:
            g3_bc = load_bc(g3_d, "g3_bc")
            bt3_bc = load_bc(bt3_d, "bt3_bc")

        # --- weights, loaded once, fp8, pre-scaled by SW on the host ---
        wqk = const.tile([128, KC_D, 2 * D], F8)
        nc.sync.dma_start(out=wqk, in_=wqk_d.rearrange("(c p) n -> p c n", p=128))
        wv = const.tile([128, 8, D], F8)
        nc.sync.dma_start(out=wv, in_=wv_d.rearrange("(c p) n -> p c n", p=128))
        pw = const.tile([128, KC_D, D], F8)
        w1t = const.tile([128, KC_D, HID], F8)
        w2t = const.tile([128, MC_H, D], F8)

        def load_big_weights():
            nc.sync.dma_start(out=pw, in_=pw_d.rearrange("(c p) n -> p c n", p=128))
            nc.sync.dma_start(out=w1t, in_=w1_d.rearrange("(c p) n -> p c n", p=128))
            nc.sync.dma_start(out=w2t, in_=w2_d.rearrange("(c p) n -> p c n", p=128))

        # --- persistent activation tiles ---
        # xnT has 8 k-tiles: 6 data + [ones-on-partition-0, zeros] for the
        # V-projection bias fold.
        xnT = const.tile([128, 8, N], F8)
        nc.gpsimd.memset(xnT[:, 6:8, :], 0.0)
        nc.gpsimd.memset(xnT[0:1, 6, :], 1.0)
        qkT2 = [const.tile([128, 12, N], F8, name=f"qkT{i}") for i in range(2)]
        ctxT2 = [const.tile([128, KC_D, N], F8, name=f"ctxT{i}") for i in range(2)]
        ffinT = const.tile([128, KC_D, N], F8)
        ffhT = const.tile([128, MC_H, N], F8)
        ffin_r = const.tile([128, TC_N, D], BF)
        # V1: per (token-chunk, head): even head -> V cols 0:64, ones col 64;
        # odd head -> ones col 0, V cols 64:128.  The softmax denominator
        # rides along the PV matmul on the aligned spare partition.
        V1 = const.tile([128, TC_N, H, 128], F8)
        nc.gpsimd.memset(V1[:, :, 0::2, 65:128], 0.0)
        nc.gpsimd.memset(V1[:, :, 1::2, 1:64], 0.0)
        nc.gpsimd.memset(V1[:, :, 0::2, 64:65], 1.0)
        nc.gpsimd.memset(V1[:, :, 1::2, 0:1], 1.0)

        # ------------------------------------------------------------------
        # Per-phase block emitters
        # ------------------------------------------------------------------
        def blk_A(b, tcn, psA=None):
            """LN1 for one token chunk -> xnT (fp8, feature-major).  Batch 0
            transposes on the idle PE (head), batch 1 via the DMA xbar (the
            s1 stretch has DVE/PE busy but idle DMA)."""
            x_t = misc.tile([128, D], F32, tag="x_in", bufs=2)
            nc.sync.dma_start(out=x_t, in_=x_d[b, tcn * 128:(tcn + 1) * 128, :])
            mean, rstd = _layer_norm(nc, misc, x_t, eps_t)
            xn_bf = misc.tile([128, D], BF, tag="xn_bf", bufs=2)
            nc.gpsimd.tensor_scalar(out=xn_bf, in0=x_t, scalar1=mean,
                                    scalar2=rstd, op0=ALU.subtract,
                                    op1=ALU.mult)
            if psA is not None:
                for kc in range(KC_D):
                    pt = psA.tile([128, 128], BF, tag="tp")
                    nc.tensor.transpose(pt, xn_bf[:, kc * 128:(kc + 1) * 128],
                                        identb)
                    nc.vector.tensor_copy(
                        out=xnT[:, kc, tcn * 128:(tcn + 1) * 128], in_=pt)
            else:
                tsc = misc.tile([128, KC_D, 128], BF, tag="tsc", bufs=3)
                nc.sync.dma_start_transpose(out=tsc, in_=xn_bf)
                nc.gpsimd.tensor_copy(
                    out=xnT[:, 0:KC_D, tcn * 128:(tcn + 1) * 128], in_=tsc)

        def blk_B(b, fc, nh, psB):
            """One q/k feature chunk for one token half."""
            ps = psB.tile([128, 512], F32, tag="qk")
            for k2 in range(3):
                nc.tensor.matmul(
                    ps,
                    wqk[:, 2 * k2:2 * k2 + 2, fc * 128:(fc + 1) * 128],
                    xnT[:, 2 * k2:2 * k2 + 2, nh * 512:(nh + 1) * 512],
                    start=(k2 == 0), stop=(k2 == 2), perf_mode=DR)
            out = qkT2[b % 2][:, fc, nh * 512:(nh + 1) * 512]
            if b % 2 == 0:
                nc.scalar.activation(out=out, in_=ps, func=AF.Identity,
                                     bias=bqk_t[:, fc:fc + 1], scale=1.0 / SW)
            else:
                nc.vector.tensor_scalar(out=out, in0=ps, scalar1=1.0 / SW,
                                        scalar2=bqk_t[:, fc:fc + 1],
                                        op0=ALU.mult, op1=ALU.add)

        def blk_C(b, tcn, psC):
            """V projection for one token chunk -> packed V1 (x SC)."""
            ps = psC.tile([128, D], F32, tag="v")
            lhs = xnT[:, :, tcn * 128:(tcn + 1) * 128]
            for k2 in range(4):
                nc.tensor.matmul(ps[:, 0:512],
                                 lhs[:, 2 * k2:2 * k2 + 2, :],
                                 wv[:, 2 * k2:2 * k2 + 2, 0:512],
                                 start=(k2 == 0), stop=(k2 == 3),
                                 perf_mode=DR)
                nc.tensor.matmul(ps[:, 512:768],
                                 lhs[:, 2 * k2:2 * k2 + 2, :],
                                 wv[:, 2 * k2:2 * k2 + 2, 512:768],
                                 start=(k2 == 0), stop=(k2 == 3),
                                 perf_mode=DR)
            pv3 = ps.rearrange("p (h d) -> p h d", d=DH)
            s = SC / SW
            if b % 2 == 0:
                nc.scalar.activation(out=V1[:, tcn, 0::2, 0:64],
                                     in_=pv3[:, 0::2, :], func=AF.Identity,
                                     scale=s)
                nc.scalar.activation(out=V1[:, tcn, 1::2, 64:128],
                                     in_=pv3[:, 1::2, :], func=AF.Identity,
                                     scale=s)
            else:
                nc.vector.tensor_scalar(out=V1[:, tcn, 0::2, 0:64],
                                        in0=pv3[:, 0::2, :], scalar1=s,
                                        scalar2=None, op0=ALU.mult)
                nc.vector.tensor_scalar(out=V1[:, tcn, 1::2, 64:128],
                                        in0=pv3[:, 1::2, :], scalar1=s,
                                        scalar2=None, op0=ALU.mult)

        def blk_D(b, nh, h, psS, psX, pPT):
            """Attention for one (query-half, head)."""
            qkT = qkT2[b % 2]
            qh = (h % 2) * 64
            dh = 64 if h % 2 == 0 else 0  # denominator partition
            fq, fk = h // 2, 6 + h // 2
            PT = pPT.tile([128, TC_N, 512], F8, tag="PT", bufs=2)
            for mc2 in range(TC_N // 2):
                sp = psS.tile([128, 1024], F32, tag="sc")
                for mi in range(2):
                    mc = mc2 * 2 + mi
                    nc.tensor.matmul(
                        sp[:, mi * 512:(mi + 1) * 512],
                        qkT[qh:qh + 64, fk, mc * 128:(mc + 1) * 128],
                        qkT[qh:qh + 64, fq, nh * 512:(nh + 1) * 512],
                        start=True, stop=True)
                nc.scalar.activation(
                    out=PT[:, mc2 * 2:mc2 * 2 + 2, :], in_=sp,
                    func=AF.Exp, scale=0.125)
            cp = psX.tile([128, 512], F32, tag="cp")
            for m2 in range(TC_N // 2):
                nc.tensor.matmul(cp, V1[:, 2 * m2:2 * m2 + 2, h, :],
                                 PT[:, 2 * m2:2 * m2 + 2, :],
                                 start=(m2 == 0), stop=(m2 == 3),
                                 perf_mode=DR)
            rd = misc.tile([1, 512], BF, tag="rd", name="rd")
            with nc.allow_low_precision(reason="bf16 recip"):
                nc.vector.reciprocal(out=rd, in_=cp[dh:dh + 1, :])
            bb = psX.tile([128, 512], F32, tag="bb")
            nc.tensor.matmul(bb, row1, rd, start=True, stop=True)
            bsb = misc.tile([128, 512], BF, tag="bbs", bufs=2)
            nc.vector.tensor_copy(out=bsb[qh:qh + 64, :], in_=bb[qh:qh + 64, :])
            nc.vector.tensor_tensor(
                out=ctxT2[b % 2][qh:qh + 64, fq, nh * 512:(nh + 1) * 512],
                in0=cp[qh:qh + 64, :], in1=bsb[qh:qh + 64, :],
                op=ALU.mult)

        def blk_E(b, tcn, psE):
            """proj + residual -> x2 -> LN2 -> ffin_r + ffinT."""
            ps = psE.tile([128, D], F32, tag="pj")
            x_t = misc.tile([128, D], F32, tag="x_in", bufs=2)
            nc.sync.dma_start(out=x_t, in_=x_d[b, tcn * 128:(tcn + 1) * 128, :])
            nc.gpsimd.tensor_tensor(out=x_t, in0=x_t, in1=pbr_bc, op=ALU.add)
            lhs = ctxT2[b % 2][:, :, tcn * 128:(tcn + 1) * 128]
            for k2 in range(3):
                nc.tensor.matmul(ps[:, 0:512],
                                 lhs[:, 2 * k2:2 * k2 + 2, :],
                                 pw[:, 2 * k2:2 * k2 + 2, 0:512],
                                 start=(k2 == 0), stop=(k2 == 2),
                                 perf_mode=DR)
                nc.tensor.matmul(ps[:, 512:768],
                                 lhs[:, 2 * k2:2 * k2 + 2, :],
                                 pw[:, 2 * k2:2 * k2 + 2, 512:768],
                                 start=(k2 == 0), stop=(k2 == 2),
                                 perf_mode=DR)
            x2 = misc.tile([128, D], F32, tag="xwork", bufs=2)
            nc.vector.scalar_tensor_tensor(
                out=x2, in0=ps, scalar=1.0 / (SW * SC), in1=x_t,
                op0=ALU.mult, op1=ALU.add)
            mean, rstd = _layer_norm(nc, misc, x2, eps_t)
            fi = ffin_r[:, tcn, :]
            nc.gpsimd.tensor_scalar(out=fi, in0=x2, scalar1=mean,
                                    scalar2=rstd, op0=ALU.subtract,
                                    op1=ALU.mult)
            if not skip_gb2:
                nc.gpsimd.tensor_tensor(out=fi, in0=fi, in1=g2_bc, op=ALU.mult)
                nc.gpsimd.tensor_tensor(out=fi, in0=fi, in1=bt2_bc, op=ALU.add)
            tsc = misc.tile([128, KC_D, 128], BF, tag="tsc", bufs=3)
            nc.sync.dma_start_transpose(out=tsc, in_=fi)
            nc.gpsimd.tensor_copy(
                out=ffinT[:, :, tcn * 128:(tcn + 1) * 128], in_=tsc)

        def blk_F(b, mc, psF, half=None):
            """fc1 + gelu for one hidden chunk (all 1024 tokens or one half)."""
            nhs = range(2) if half is None else [half]
            w = N if half is None else 512
            ps = psF.tile([128, w], F32, tag="f1")
            for k2 in range(3):
                w1s = w1t[:, 2 * k2:2 * k2 + 2, mc * 128:(mc + 1) * 128]
                for j, nh in enumerate(nhs):
                    nc.tensor.matmul(
                        ps[:, j * 512:(j + 1) * 512],
                        w1s,
                        ffinT[:, 2 * k2:2 * k2 + 2, nh * 512:(nh + 1) * 512],
                        start=(k2 == 0), stop=(k2 == 2),
                        perf_mode=DR)
            out = ffhT[:, mc, :] if half is None else \
                ffhT[:, mc, half * 512:(half + 1) * 512]
            if b % 2 == 0:
                # gelu(x) ~ 0.5x(1+tanh(0.851x)); Tanh shares the act table
                # with Exp so interleaving with softmax causes no table swaps.
                th = misc.tile([128, w], BF, tag="tnh", bufs=2)
                nc.scalar.activation(out=th, in_=ps, func=AF.Tanh,
                                     bias=b1a_t[:, mc:mc + 1], scale=0.851 / SW)
                u = misc.tile([128, w], BF, tag="uu", bufs=2)
                if mc % 2 == 0:
                    nc.scalar.activation(out=u, in_=ps, func=AF.Identity,
                                         bias=b1h_t[:, mc:mc + 1],
                                         scale=0.5 / SW)
                else:
                    nc.vector.tensor_scalar(out=u, in0=ps, scalar1=0.5 / SW,
                                            scalar2=b1h_t[:, mc:mc + 1],
                                            op0=ALU.mult, op1=ALU.add)
                nc.vector.scalar_tensor_tensor(out=out, in0=th,
                                               scalar=1.0, in1=u,
                                               op0=ALU.add, op1=ALU.mult)
            else:
                # tail: no softmax exps left, exact gelu costs one table load
                nc.scalar.activation(out=out, in_=ps, func=AF.Gelu,
                                     bias=b1_t[:, mc:mc + 1], scale=1.0 / SW)

        def blk_G(b, ti, psG):
            """fc2 + residual + LN3 -> y."""
            ps = psG.tile([128, D], F32, tag="f2")
            lhs = ffhT[:, :, ti * 128:(ti + 1) * 128]
            for k2 in range(MC_H // 2):
                nc.tensor.matmul(ps[:, 0:512],
                                 lhs[:, 2 * k2:2 * k2 + 2, :],
                                 w2t[:, 2 * k2:2 * k2 + 2, 0:512],
                                 start=(k2 == 0), stop=(k2 == 11),
                                 perf_mode=DR)
                nc.tensor.matmul(ps[:, 512:768],
                                 lhs[:, 2 * k2:2 * k2 + 2, :],
                                 w2t[:, 2 * k2:2 * k2 + 2, 512:768],
                                 start=(k2 == 0), stop=(k2 == 11),
                                 perf_mode=DR)
            x3 = misc.tile([128, D], F32, tag="xwork", bufs=2)
            nc.vector.scalar_tensor_tensor(
                out=x3, in0=ps, scalar=1.0 / SW, in1=ffin_r[:, ti, :],
                op0=ALU.mult, op1=ALU.add)
            nc.gpsimd.tensor_tensor(out=x3, in0=x3, in1=b2_bc, op=ALU.add)
            mean, rstd = _layer_norm(nc, misc, x3, eps_t)
            yt = misc.tile([128, D], F32, tag="xout", bufs=2)
            nc.gpsimd.tensor_scalar(out=yt, in0=x3, scalar1=mean,
                                    scalar2=rstd,
                                    op0=ALU.subtract, op1=ALU.mult)
            if not skip_gb3:
                nc.gpsimd.tensor_tensor(out=yt, in0=yt, in1=g3_bc, op=ALU.mult)
                nc.gpsimd.tensor_tensor(out=yt, in0=yt, in1=bt3_bc, op=ALU.add)
            nc.sync.dma_start(out=y_d[b, ti * 128:(ti + 1) * 128, :], in_=yt)

        # ------------------------------------------------------------------
        # Emission schedule (software pipeline over the BLOC=2 batches).
        # ------------------------------------------------------------------
        def interleave(primary, secondary, lead=1.25):
            """Emit primary blocks with secondary blocks spread between,
            front-loading the secondaries slightly so their downstream engine
            work drains before the primary stretch ends."""
            np_, ns = len(primary), len(secondary)
            si = 0
            for i, p in enumerate(primary):
                p()
                want = min(ns, int((i + 1) * ns * lead / np_))
                while si < want:
                    secondary[si]()
                    si += 1
            while si < ns:
                secondary[si]()
                si += 1

        b_pair_order = [(fc, nh) for p in range(6)
                        for fc in (p, 6 + p) for nh in range(2)]

        for rep in range(reps):
            r0, r1 = 2 * rep, 2 * rep + 1

            # -- head: A0 with B0/C0 interleaved per token-half --
            psA0 = tc.alloc_tile_pool(name=f"psA{r0}", bufs=2, space="PSUM")
            psB0 = tc.alloc_tile_pool(name=f"psB{r0}", bufs=2, space="PSUM")
            psC0 = tc.alloc_tile_pool(name=f"psC{r0}", bufs=2, space="PSUM")
            for tcn in range(4):
                blk_A(0, tcn, psA0)
            pri = [(lambda t=t: blk_A(0, t, psA0)) for t in range(4, TC_N)]
            pri += [(lambda t=t: blk_C(0, t, psC0)) for t in range(4)]
            sec = [(lambda fc=fc, nh=nh: blk_B(0, fc, nh, psB0))
                   for fc, nh in b_pair_order if nh == 0]
            interleave(pri, sec)
            pri = [(lambda t=t: blk_C(0, t, psC0)) for t in range(4, TC_N)]
            sec = [(lambda fc=fc, nh=nh: blk_B(0, fc, nh, psB0))
                   for fc, nh in b_pair_order if nh == 1]
            interleave(pri, sec)
            psC0.release()
            psB0.release()
            psA0.release()

            # -- stretch 1: D0 (ACT-bound) over A1 B1 --
            psS0 = tc.alloc_tile_pool(name=f"psS{r0}", bufs=2, space="PSUM")
            psX0 = tc.alloc_tile_pool(name=f"psX{r0}", bufs=1, space="PSUM")
            pPT0 = tc.alloc_tile_pool(name=f"pPT{r0}", bufs=2)
            psB1 = tc.alloc_tile_pool(name=f"psB{r1}", bufs=1, space="PSUM")
            sec = [load_big_weights]
            sec += [(lambda t=t: blk_A(1, t)) for t in range(TC_N)]
            sec += [(lambda fc=fc, nh=nh: blk_B(1, fc, nh, psB1))
                    for fc, nh in b_pair_order]
            pri = [(lambda nh=nh, h=h: blk_D(0, nh, h, psS0, psX0, pPT0))
                   for nh in range(2) for h in range(H)]
            interleave(pri, sec)
            psB1.release()
            psX0.release()
            psS0.release()
            pPT0.release()

            # -- stretch 2 psum reserved first so D1 scores/exp overlap C1 --
            psS1 = tc.alloc_tile_pool(name=f"psS{r1}", bufs=2, space="PSUM")
            psX1 = tc.alloc_tile_pool(name=f"psX{r1}", bufs=1, space="PSUM")
            pPT1 = tc.alloc_tile_pool(name=f"pPT{r1}", bufs=2)

            # -- gap: C1 (V projection for batch 1) --
            psC1 = tc.alloc_tile_pool(name=f"psC{r1}", bufs=1, space="PSUM")
            for tcn in range(TC_N):
                blk_C(1, tcn, psC1)
            psC1.release()
            efg_pools = {}

            def efg_open(key, bufs=1):
                efg_pools[key] = tc.alloc_tile_pool(
                    name=f"ps{key}{r0}", bufs=bufs, space="PSUM")

            sec = [lambda: efg_open("E")]
            sec += [(lambda t=t: blk_E(0, t, efg_pools["E"])) for t in range(TC_N)]
            sec += [lambda: efg_pools["E"].release(), lambda: efg_open("F")]

            def f_burst(b, mc0):
                for mc in range(mc0, mc0 + 8):
                    blk_F(b, mc, efg_pools["F"])

            sec += [(lambda m=m: f_burst(0, m)) for m in range(0, MC_H, 8)]
            sec += [lambda: efg_pools["F"].release(), lambda: efg_open("G")]
            sec += [(lambda t=t: blk_G(0, t, efg_pools["G"])) for t in range(TC_N)]
            sec += [lambda: efg_pools["G"].release(), lambda: efg_open("E1", 1)]
            sec += [(lambda t=t: blk_E(1, t, efg_pools["E1"])) for t in range(4)]
            pri = [(lambda nh=nh, h=h: blk_D(1, nh, h, psS1, psX1, pPT1))
                   for nh in range(2) for h in range(H)]
            interleave(pri, sec)
            efg_pools["E1"].release()
            psX1.release()
            psS1.release()
            pPT1.release()

            # -- tail: E1 F1 G1 --
            psE1 = tc.alloc_tile_pool(name=f"psE{r1}b", bufs=3, space="PSUM")
            for tcn in range(4, TC_N):
                blk_E(1, tcn, psE1)
            psE1.release()
            psF1 = tc.alloc_tile_pool(name=f"psF{r1}", bufs=3, space="PSUM")
            for mc in range(MC_H):
                blk_F(1, mc, psF1)
            psF1.release()
            psG1 = tc.alloc_tile_pool(name=f"psG{r1}", bufs=3, space="PSUM")
            for ti in range(TC_N):
                blk_G(1, ti, psG1)
            psG1.release()

        const.release()
        misc.release()

    _split_sync_waits(nc)
    return nc


_NC_CACHE = {}


def _get_nc(reps=1, skip_gb2=False, skip_gb3=False):
    key = f"nc{reps}_{skip_gb2}_{skip_gb3}"
    if key not in _NC_CACHE:
        _NC_CACHE[key] = _build_nc(reps, skip_gb2, skip_gb3)
    return _NC_CACHE[key]


def kernel(x, ln1_g, ln1_b, qkv_w, qkv_b, proj_w, proj_b,
           ln2_g, ln2_b, fc1_w, fc1_b, fc2_w, fc2_b, ln3_g, ln3_b,
           **extra):
    x = np.ascontiguousarray(np.asarray(x, np.float32))
    f = lambda a: np.ascontiguousarray(np.asarray(a, np.float32))
    f8 = lambda a: np.ascontiguousarray(np.asarray(SW * a, E4NP))
    qkv_w, qkv_b = f(qkv_w), f(qkv_b)

    # Fold LN1 gamma/beta into QKV weights/bias (host, fp32).
    w_eff = np.asarray(ln1_g, np.float32)[:, None] * qkv_w
    b_eff = np.asarray(ln1_b, np.float32) @ qkv_w + qkv_b

    # V weights: 6 k-tile chunks of SW*w_v + a bias k-tile pair whose first
    # row is SW*b_v (multiplied on-device by the ones row in xnT k-tile 6).
    wv_pack = np.zeros((8 * 128, D), np.float32)
    wv_pack[:D] = SW * w_eff[:, 2 * D:]
    wv_pack[D] = SW * b_eff[2 * D:]

    common = {
        "w_qk": f8(w_eff[:, :2 * D]),
        "w_v": np.ascontiguousarray(np.asarray(wv_pack, E4NP)),
        "b_qk": f(b_eff[:2 * D]),
        "p_w": f8(proj_w), "pbr": f(proj_b),
        "w1": f8(fc1_w), "b1": f(fc1_b),
        "w2": f8(fc2_w), "b2": f(fc2_b),
        "g2": f(ln2_g), "bt2": f(ln2_b),
        "g3": f(ln3_g), "bt3": f(ln3_b),
    }
    in_maps = [dict(common, x=x[i * BLOC:(i + 1) * BLOC]) for i in range(NCORES)]

    skip_gb2 = bool(np.all(common["g2"] == 1.0) and np.all(common["bt2"] == 0.0))
    skip_gb3 = bool(np.all(common["g3"] == 1.0) and np.all(common["bt3"] == 0.0))
    nc = _get_nc(1, skip_gb2, skip_gb3)
    res = run_bass_kernel_spmd(nc, in_maps, core_ids=list(range(NCORES)))
    _NC_CACHE["last_result"] = res
    return np.concatenate([r["y"] for r in res.results], axis=0)


# revision 39
# speedup vs baseline: 1.0160x; 1.0082x over previous
"""EncoderBlock kernel for 8 Trainium2 NeuronCores (data-parallel over batch).

Contract: kernel(**inputs) takes the FULL inputs of reference.setup_inputs()
and returns the FULL [16, 1024, 768] float32 output.

Strategy: pure data parallelism — 16 batches / 8 cores = 2 batches per core,
weights replicated, zero collectives.  Per core a fused Bass/Tile program runs
LN1 -> QKV -> attention -> proj -> residual -> LN2 -> fc1/gelu -> fc2 ->
(normalized residual) -> LN3.

Precision: all large GEMMs use fp8e4 (e4m3) operands with
perf_mode=DoubleRow (two 128-deep k-tiles contracted per instruction);
weights are pre-scaled by 512 on the host so their magnitudes sit in fp8's
normal range, and the 1/512 unscale folds into the downstream evacuation
ops (or the exp/gelu activation scale).  LayerNorm stats, softmax
denominators and residual adds stay fp32/bf16.  Host-emulated end-to-end
error: ~1.1e-2 max-rel vs the fp32 reference (gate 2e-2).

Schedule: the two batches are software-pipelined so batch1's LN1/QKV/V
(PE+DVE+Pool) runs under batch0's ACT-bound softmax stretch, and batch0's
proj/MLP runs under batch1's softmax stretch.  qkT/ctxT are double-buffered
across batches to allow the overlap.  LN rsqrt runs as exp(-0.5*ln(var+eps))
and the overlapped-batch gelu as 0.5x(1+tanh(0.851x)) so every ACT function
in the busy stretches shares one activation table with the softmax Exp
(act-table reloads cost 1.3us each and the Tile scheduler freely interleaves
ACT work).  Transposes ride the DMA xbar (batch1) or the idle head-phase PE
(batch0).  Cost-model time: ~398us vs ~742us for the fp32r baseline.
"""

import os
import sys

sys.path.insert(0, "/opt/trn_rl_repo")
# The axon NTFF profiling hook is unavailable in this environment; force
# tracing off so an externally-set BASS_TRACE cannot break execution.
os.environ["BASS_NEVER_TRACE"] = "1"

import numpy as np
import ml_dtypes

import concourse.bass as bass
import concourse.tile as tile
from concourse import mybir
from concourse.vector_clock import ScopedClock, VectorClock
from concourse.bass_utils import run_bass_kernel_spmd

F32 = mybir.dt.float32
BF = mybir.dt.bfloat16
F8 = mybir.dt.float8e4
AF = mybir.ActivationFunctionType
ALU = mybir.AluOpType
DR = mybir.MatmulPerfMode.DoubleRow

B, N, D = 16, 1024, 768
H, DH, HID = 12, 64, 3072
NCORES = 8
BLOC = B // NCORES
EPS = 1e-5
TC_N = N // 128   # 8 token tiles / batch
KC_D = D // 128   # 6 feature chunks
MC_H = HID // 128  # 24 hidden chunks
SW = 512.0        # host weight prescale (folded out after each GEMM)
SC = 32.0         # ctx scale, folded into the V values

E4NP = ml_dtypes.float8_e4m3


# ---------------------------------------------------------------------------
# Workarounds: this walrus build rejects >1 sync-wait command per instruction.
# ---------------------------------------------------------------------------
def _patched_drain_and_barrier(self, tick_clock, wait_clock):
    gc = tick_clock.global_clock
    n = len(gc)
    for i in range(n):
        t = gc[i]
        if t <= 0:
            continue
        vec = [0] * n
        vec[i] = t
        nop = self.nc.sync.nop(nofuse=True)
        wait_clock.add_sem_waits(nop.ins, ScopedClock({None: VectorClock(vec)}))
    self.nc.sync.drain()
    self.nc.all_engine_barrier()
    assert self.sems is not None
    popped = self.nc._tile_sem_poison_stack.pop()
    assert popped is self._sem_poison
    self.nc.clear_and_free_semaphores(list(self.sems.allocated().values()))
    self.nc.all_engine_barrier()


tile.TileContext._drain_and_barrier = _patched_drain_and_barrier


def _split_sync_waits(nc, limit=1):
    """Move excess per-instruction sync waits onto same-engine NoOps."""
    n_split = 0
    for fn in nc.m.functions:
        for bb in fn.blocks:
            out = []
            for ins in bb.instructions:
                si = ins.sync_info
                waits = list(si.on_wait) if (si and si.on_wait) else []
                if len(waits) > limit:
                    excess, keep = waits[:-limit], waits[-limit:]
                    for w in excess:
                        nop = mybir.InstNoOp(
                            name=f"{ins.name}-ws{n_split}",
                            engine=ins.engine,
                            ins=[],
                            outs=[],
                            sync_info=mybir.SyncInfo(on_wait=[w], on_update=[]),
                        )
                        n_split += 1
                        out.append(nop)
                    si.on_wait = keep
                out.append(ins)
            bb.instructions = out
    return n_split


def _layer_norm(nc, misc, x_t, eps_t):
    """(mean, rstd) [128,1] via DVE bn_stats for x_t [128, 768].  The rsqrt
    is exp(-0.5*ln(var+eps)) on ACT: Ln and Exp share an act table with the
    softmax Exp, so LayerNorms cause no act-table swaps."""
    xr = x_t.rearrange("p (s d) -> p s d", d=256)
    lt = misc.tile([128, 24], F32, tag="lnb", bufs=3)
    stats = lt[:, 0:18].rearrange("p (s d) -> p s d", d=6)
    for s in range(3):
        nc.vector.bn_stats(out=stats[:, s, :], in_=xr[:, s, :])
    mv = lt[:, 18:20]
    nc.vector.bn_aggr(out=mv, in_=lt[:, 0:18])
    lv = lt[:, 20:21]
    nc.scalar.activation(out=lv, in_=mv[:, 1:2], func=AF.Ln, bias=eps_t)
    rstd = misc.tile([128, 1], F32, tag="rstd", bufs=3)
    nc.scalar.activation(out=rstd, in_=lv, func=AF.Exp, scale=-0.5)
    return mv[:, 0:1], rstd


def _build_nc(reps=1, skip_gb2=False, skip_gb3=False):
    nc = bass.Bass()

    x_d = nc.dram_tensor("x", [BLOC, N, D], F32, kind="ExternalInput")
    wqk_d = nc.dram_tensor("w_qk", [D, 2 * D], F8, kind="ExternalInput")
    # wv_d carries 8 k-tiles: 6 weight chunks + [bias row, zeros] pair
    wv_d = nc.dram_tensor("w_v", [8 * 128, D], F8, kind="ExternalInput")
    bqk_d = nc.dram_tensor("b_qk", [2 * D], F32, kind="ExternalInput")
    pw_d = nc.dram_tensor("p_w", [D, D], F8, kind="ExternalInput")
    pbr_d = nc.dram_tensor("pbr", [D], F32, kind="ExternalInput")
    w1_d = nc.dram_tensor("w1", [D, HID], F8, kind="ExternalInput")
    b1_d = nc.dram_tensor("b1", [HID], F32, kind="ExternalInput")
    w2_d = nc.dram_tensor("w2", [HID, D], F8, kind="ExternalInput")
    b2_d = nc.dram_tensor("b2", [D], F32, kind="ExternalInput")
    g2_d = nc.dram_tensor("g2", [D], F32, kind="ExternalInput")
    bt2_d = nc.dram_tensor("bt2", [D], F32, kind="ExternalInput")
    g3_d = nc.dram_tensor("g3", [D], F32, kind="ExternalInput")
    bt3_d = nc.dram_tensor("bt3", [D], F32, kind="ExternalInput")
    y_d = nc.dram_tensor("y", [BLOC, N, D], F32, kind="ExternalOutput")

    with tile.TileContext(nc, pool_alloc_mode="queue") as tc:
        misc = tc.alloc_tile_pool(name="misc", bufs=2)
        const = tc.alloc_tile_pool(name="const", bufs=1)

        eps_t = const.tile([128, 1], F32)
        nc.vector.memset(eps_t, EPS)
        bqk_t = const.tile([128, 12], F32)
        nc.sync.dma_start(out=bqk_t, in_=bqk_d.rearrange("(c p) -> p c", p=128))
        b1_t = const.tile([128, MC_H], F32)
        nc.sync.dma_start(out=b1_t, in_=b1_d.rearrange("(c p) -> p c", p=128))
        # gelu-via-tanh biases: 0.851*b1 (tanh arg) and 0.5*b1 (linear part)
        b1a_t = const.tile([128, MC_H], F32)
        nc.gpsimd.tensor_scalar(out=b1a_t, in0=b1_t, scalar1=0.851,
                                scalar2=None, op0=ALU.mult)
        b1h_t = const.tile([128, MC_H], F32)
        nc.gpsimd.tensor_scalar(out=b1h_t, in0=b1_t, scalar1=0.5,
                                scalar2=None, op0=ALU.mult)
        row1 = const.tile([1, 128], BF)
        nc.vector.memset(row1, 1.0)
        identb = const.tile([128, 128], BF)
        from concourse.masks import make_identity
        make_identity(nc, identb)

        def load_bc(dd, nm):
            t = const.tile([128, D], F32, name=nm)
            nc.sync.dma_start(out=t, in_=dd[None, :].partition_broadcast(128))
            return t

        pbr_bc = load_bc(pbr_d, "pbr_bc")
        b2_bc = load_bc(b2_d, "b2_bc")
        if not skip_gb2:
            g2_bc = load_bc(g2_d, "g2_bc")
            bt2_bc = load_bc(bt2_d, "bt2_bc")
        if not skip_gb3:
            g3_bc = load_bc(g3_d, "g3_bc")
            bt3_bc = load_bc(bt3_d, "bt3_bc")

        # --- weights, loaded once, fp8, pre-scaled by SW on the host ---
        wqk = const.tile([128, KC_D, 2 * D], F8)
        nc.sync.dma_start(out=wqk, in_=wqk_d.rearrange("(c p) n -> p c n", p=128))
        wv = const.tile([128, 8, D], F8)
        nc.sync.dma_start(out=wv, in_=wv_d.rearrange("(c p) n -> p c n", p=128))
        pw = const.tile([128, KC_D, D], F8)
        w1t = const.tile([128, KC_D, HID], F8)
        w2t = const.tile([128, MC_H, D], F8)

        def load_big_weights():
            nc.sync.dma_start(out=pw, in_=pw_d.rearrange("(c p) n -> p c n", p=128))
            nc.sync.dma_start(out=w1t, in_=w1_d.rearrange("(c p) n -> p c n", p=128))
            nc.sync.dma_start(out=w2t, in_=w2_d.rearrange("(c p) n -> p c n", p=128))

        # --- persistent activation tiles ---
        # xnT has 8 k-tiles: 6 data + [ones-on-partition-0, zeros] for the
        # V-projection bias fold.
        xnT = const.tile([128, 8, N], F8)
        nc.gpsimd.memset(xnT[:, 6:8, :], 0.0)
        nc.gpsimd.memset(xnT[0:1, 6, :], 1.0)
        qkT2 = [const.tile([128, 12, N], F8, name=f"qkT{i}") for i in range(2)]
        ctxT2 = [const.tile([128, KC_D, N], F8, name=f"ctxT{i}") for i in range(2)]
        ffinT = const.tile([128, KC_D, N], F8)
        ffhT = const.tile([128, MC_H, N], F8)
        ffin_r = const.tile([128, TC_N, D], BF)
        # V1: per (token-chunk, head): even head -> V cols 0:64, ones col 64;
        # odd head -> ones col 0, V cols 64:128.  The softmax denominator
        # rides along the PV matmul on the aligned spare partition.
        V1 = const.tile([128, TC_N, H, 128], F8)
        nc.gpsimd.memset(V1[:, :, 0::2, 65:128], 0.0)
        nc.gpsimd.memset(V1[:, :, 1::2, 1:64], 0.0)
        nc.gpsimd.memset(V1[:, :, 0::2, 64:65], 1.0)
        nc.gpsimd.memset(V1[:, :, 1::2, 0:1], 1.0)

        # ------------------------------------------------------------------
        # Per-phase block emitters
        # ------------------------------------------------------------------
        def blk_A(b, tcn, psA=None):
            """LN1 for one token chunk -> xnT (fp8, feature-major).  Batch 0
            transposes on the idle PE (head), batch 1 via the DMA xbar (the
            s1 stretch has DVE/PE busy but idle DMA)."""
            x_t = misc.tile([128, D], F32, tag="x_in", bufs=2)
            nc.sync.dma_start(out=x_t, in_=x_d[b, tcn * 128:(tcn + 1) * 128, :])
            mean, rstd = _layer_norm(nc, misc, x_t, eps_t)
            xn_bf = misc.tile([128, D], BF, tag="xn_bf", bufs=2)
            nc.gpsimd.tensor_scalar(out=xn_bf, in0=x_t, scalar1=mean,
                                    scalar2=rstd, op0=ALU.subtract,
                                    op1=ALU.mult)
            if psA is not None:
                for kc in range(KC_D):
                    pt = psA.tile([128, 128], BF, tag="tp")
                    nc.tensor.transpose(pt, xn_bf[:, kc * 128:(kc + 1) * 128],
                                        identb)
                    nc.vector.tensor_copy(
                        out=xnT[:, kc, tcn * 128:(tcn + 1) * 128], in_=pt)
            else:
                tsc = misc.tile([128, KC_D, 128], BF, tag="tsc", bufs=3)
                nc.sync.dma_start_transpose(out=tsc, in_=xn_bf)
                nc.gpsimd.tensor_copy(
                    out=xnT[:, 0:KC_D, tcn * 128:(tcn + 1) * 128], in_=tsc)

        def blk_B(b, fc, nh, psB):
            """One q/k feature chunk for one token half."""
            ps = psB.tile([128, 512], F32, tag="qk")
            for k2 in range(3):
                nc.tensor.matmul(
                    ps,
                    wqk[:, 2 * k2:2 * k2 + 2, fc * 128:(fc + 1) * 128],
                    xnT[:, 2 * k2:2 * k2 + 2, nh * 512:(nh + 1) * 512],
                    start=(k2 == 0), stop=(k2 == 2), perf_mode=DR)
            out = qkT2[b % 2][:, fc, nh * 512:(nh + 1) * 512]
            if b % 2 == 0:
                nc.scalar.activation(out=out, in_=ps, func=AF.Identity,
                                     bias=bqk_t[:, fc:fc + 1], scale=1.0 / SW)
            else:
                nc.vector.tensor_scalar(out=out, in0=ps, scalar1=1.0 / SW,
                                        scalar2=bqk_t[:, fc:fc + 1],
                                        op0=ALU.mult, op1=ALU.add)

        def blk_C(b, tcn, psC):
            """V projection for one token chunk -> packed V1 (x SC)."""
            ps = psC.tile([128, D], F32, tag="v")
            lhs = xnT[:, :, tcn * 128:(tcn + 1) * 128]
            for k2 in range(4):
                nc.tensor.matmul(ps[:, 0:512],
                                 lhs[:, 2 * k2:2 * k2 + 2, :],
                                 wv[:, 2 * k2:2 * k2 + 2, 0:512],
                                 start=(k2 == 0), stop=(k2 == 3),
                                 perf_mode=DR)
                nc.tensor.matmul(ps[:, 512:768],
                                 lhs[:, 2 * k2:2 * k2 + 2, :],
                                 wv[:, 2 * k2:2 * k2 + 2, 512:768],
                                 start=(k2 == 0), stop=(k2 == 3),
                                 perf_mode=DR)
            pv3 = ps.rearrange("p (h d) -> p h d", d=DH)
            s = SC / SW
            if b % 2 == 0:
                nc.scalar.activation(out=V1[:, tcn, 0::2, 0:64],
                                     in_=pv3[:, 0::2, :], func=AF.Identity,
                                     scale=s)
                nc.scalar.activation(out=V1[:, tcn, 1::2, 64:128],
                                     in_=pv3[:, 1::2, :], func=AF.Identity,
                                     scale=s)
            else:
                nc.vector.tensor_scalar(out=V1[:, tcn, 0::2, 0:64],
                                        in0=pv3[:, 0::2, :], scalar1=s,
                                        scalar2=None, op0=ALU.mult)
                nc.vector.tensor_scalar(out=V1[:, tcn, 1::2, 64:128],
                                        in0=pv3[:, 1::2, :], scalar1=s,
                                        scalar2=None, op0=ALU.mult)

        def blk_D(b, nh, h, psS, psX, pPT):
            """Attention for one (query-half, head)."""
            qkT = qkT2[b % 2]
            qh = (h % 2) * 64
            dh = 64 if h % 2 == 0 else 0  # denominator partition
            fq, fk = h // 2, 6 + h // 2
            PT = pPT.tile([128, TC_N, 512], F8, tag="PT", bufs=2)
            for mc2 in range(TC_N // 2):
                sp = psS.tile([128, 1024], F32, tag="sc")
                for mi in range(2):
                    mc = mc2 * 2 + mi
                    nc.tensor.matmul(
                        sp[:, mi * 512:(mi + 1) * 512],
                        qkT[qh:qh + 64, fk, mc * 128:(mc + 1) * 128],
                        qkT[qh:qh + 64, fq, nh * 512:(nh + 1) * 512],
                        start=True, stop=True)
                nc.scalar.activation(
                    out=PT[:, mc2 * 2:mc2 * 2 + 2, :], in_=sp,
                    func=AF.Exp, scale=0.125)
            cp = psX.tile([128, 512], F32, tag="cp")
            for m2 in range(TC_N // 2):
                nc.tensor.matmul(cp, V1[:, 2 * m2:2 * m2 + 2, h, :],
                                 PT[:, 2 * m2:2 * m2 + 2, :],
                                 start=(m2 == 0), stop=(m2 == 3),
                                 perf_mode=DR)
            rd = misc.tile([1, 512], BF, tag="rd", name="rd")
            with nc.allow_low_precision(reason="bf16 recip"):
                nc.vector.reciprocal(out=rd, in_=cp[dh:dh + 1, :])
            bb = psX.tile([128, 512], F32, tag="bb")
            nc.tensor.matmul(bb, row1, rd, start=True, stop=True)
            bsb = misc.tile([128, 512], BF, tag="bbs", bufs=2)
            nc.vector.tensor_copy(out=bsb[qh:qh + 64, :], in_=bb[qh:qh + 64, :])
            nc.vector.tensor_tensor(
                out=ctxT2[b % 2][qh:qh + 64, fq, nh * 512:(nh + 1) * 512],
                in0=cp[qh:qh + 64, :], in1=bsb[qh:qh + 64, :],
                op=ALU.mult)

        def blk_E(b, tcn, psE):
            """proj + residual -> x2 -> LN2 -> ffin_r + ffinT."""
            ps = psE.tile([128, D], F32, tag="pj")
            x_t = misc.tile([128, D], F32, tag="x_in", bufs=2)
            nc.sync.dma_start(out=x_t, in_=x_d[b, tcn * 128:(tcn + 1) * 128, :])
            nc.gpsimd.tensor_tensor(out=x_t, in0=x_t, in1=pbr_bc, op=ALU.add)
            lhs = ctxT2[b % 2][:, :, tcn * 128:(tcn + 1) * 128]
            for k2 in range(3):
                nc.tensor.matmul(ps[:, 0:512],
                                 lhs[:, 2 * k2:2 * k2 + 2, :],
                                 pw[:, 2 * k2:2 * k2 + 2, 0:512],
                                 start=(k2 == 0), stop=(k2 == 2),
                                 perf_mode=DR)
                nc.tensor.matmul(ps[:, 512:768],
                                 lhs[:, 2 * k2:2 * k2 + 2, :],
                                 pw[:, 2 * k2:2 * k2 + 2, 512:768],
                                 start=(k2 == 0), stop=(k2 == 2),
                                 perf_mode=DR)
            x2 = misc.tile([128, D], F32, tag="xwork", bufs=2)
            nc.vector.scalar_tensor_tensor(
                out=x2, in0=ps, scalar=1.0 / (SW * SC), in1=x_t,
                op0=ALU.mult, op1=ALU.add)
            mean, rstd = _layer_norm(nc, misc, x2, eps_t)
            fi = ffin_r[:, tcn, :]
            nc.gpsimd.tensor_scalar(out=fi, in0=x2, scalar1=mean,
                                    scalar2=rstd, op0=ALU.subtract,
                                    op1=ALU.mult)
            if not skip_gb2:
                nc.gpsimd.tensor_tensor(out=fi, in0=fi, in1=g2_bc, op=ALU.mult)
                nc.gpsimd.tensor_tensor(out=fi, in0=fi, in1=bt2_bc, op=ALU.add)
            tsc = misc.tile([128, KC_D, 128], BF, tag="tsc", bufs=3)
            nc.sync.dma_start_transpose(out=tsc, in_=fi)
            nc.gpsimd.tensor_copy(
                out=ffinT[:, :, tcn * 128:(tcn + 1) * 128], in_=tsc)

        def blk_F(b, mc, psF, half=None):
            """fc1 + gelu for one hidden chunk (all 1024 tokens or one half)."""
            nhs = range(2) if half is None else [half]
            w = N if half is None else 512
            ps = psF.tile([128, w], F32, tag="f1")
            for k2 in range(3):
                w1s = w1t[:, 2 * k2:2 * k2 + 2, mc * 128:(mc + 1) * 128]
                for j, nh in enumerate(nhs):
                    nc.tensor.matmul(
                        ps[:, j * 512:(j + 1) * 512],
                        w1s,
                        ffinT[:, 2 * k2:2 * k2 + 2, nh * 512:(nh + 1) * 512],
                        start=(k2 == 0), stop=(k2 == 2),
                        perf_mode=DR)
            out = ffhT[:, mc, :] if half is None else \
                ffhT[:, mc, half * 512:(half + 1) * 512]
            if b % 2 == 0:
                # gelu(x) ~ 0.5x(1+tanh(0.851x)); Tanh shares the act table
                # with Exp so interleaving with softmax causes no table swaps.
                th = misc.tile([128, w], BF, tag="tnh", bufs=2)
                nc.scalar.activation(out=th, in_=ps, func=AF.Tanh,
                                     bias=b1a_t[:, mc:mc + 1], scale=0.851 / SW)
                u = misc.tile([128, w], BF, tag="uu", bufs=2)
                if mc % 2 == 0:
                    nc.scalar.activation(out=u, in_=ps, func=AF.Identity,
                                         bias=b1h_t[:, mc:mc + 1],
                                         scale=0.5 / SW)
                else:
                    nc.vector.tensor_scalar(out=u, in0=ps, scalar1=0.5 / SW,
                                            scalar2=b1h_t[:, mc:mc + 1],
                                            op0=ALU.mult, op1=ALU.add)
                nc.vector.scalar_tensor_tensor(out=out, in0=th,
                                               scalar=1.0, in1=u,
                                               op0=ALU.add, op1=ALU.mult)
            else:
                # tail: no softmax exps left, exact gelu costs one table load
                nc.scalar.activation(out=out, in_=ps, func=AF.Gelu,
                                     bias=b1_t[:, mc:mc + 1], scale=1.0 / SW)

        def blk_G(b, ti, psG):
            """fc2 + residual + LN3 -> y."""
            ps = psG.tile([128, D], F32, tag="f2")
            lhs = ffhT[:, :, ti * 128:(ti + 1) * 128]
            for k2 in range(MC_H // 2):
                nc.tensor.matmul(ps[:, 0:512],
                                 lhs[:, 2 * k2:2 * k2 + 2, :],
                                 w2t[:, 2 * k2:2 * k2 + 2, 0:512],
                                 start=(k2 == 0), stop=(k2 == 11),
                                 perf_mode=DR)
                nc.tensor.matmul(ps[:, 512:768],
                                 lhs[:, 2 * k2:2 * k2 + 2, :],
                                 w2t[:, 2 * k2:2 * k2 + 2, 512:768],
                                 start=(k2 == 0), stop=(k2 == 11),
                                 perf_mode=DR)
            x3 = misc.tile([128, D], F32, tag="xwork", bufs=2)
            nc.vector.scalar_tensor_tensor(
                out=x3, in0=ps, scalar=1.0 / SW, in1=ffin_r[:, ti, :],
                op0=ALU.mult, op1=ALU.add)
            nc.gpsimd.tensor_tensor(out=x3, in0=x3, in1=b2_bc, op=ALU.add)
            mean, rstd = _layer_norm(nc, misc, x3, eps_t)
            yt = misc.tile([128, D], F32, tag="xout", bufs=2)
            nc.gpsimd.tensor_scalar(out=yt, in0=x3, scalar1=mean,
                                    scalar2=rstd,
                                    op0=ALU.subtract, op1=ALU.mult)
            if not skip_gb3:
                nc.gpsimd.tensor_tensor(out=yt, in0=yt, in1=g3_bc, op=ALU.mult)
                nc.gpsimd.tensor_tensor(out=yt, in0=yt, in1=bt3_bc, op=ALU.add)
            nc.sync.dma_start(out=y_d[b, ti * 128:(ti + 1) * 128, :], in_=yt)

        # ------------------------------------------------------------------
        # Emission schedule (software pipeline over the BLOC=2 batches).
        # ------------------------------------------------------------------
        def interleave(primary, secondary, lead=1.5):
            """Emit primary blocks with secondary blocks spread between,
            front-loading the secondaries slightly so their downstream engine
            work drains before the primary stretch ends."""
            np_, ns = len(primary), len(secondary)
            si = 0
            for i, p in enumerate(primary):
                p()
                want = min(ns, int((i + 1) * ns * lead / np_))
                while si < want:
                    secondary[si]()
                    si += 1
            while si < ns:
                secondary[si]()
                si += 1

        b_pair_order = [(fc, nh) for p in range(6)
                        for fc in (p, 6 + p) for nh in range(2)]

        for rep in range(reps):
            r0, r1 = 2 * rep, 2 * rep + 1

            # -- head: A0 with B0/C0 interleaved per token-half --
            psA0 = tc.alloc_tile_pool(name=f"psA{r0}", bufs=2, space="PSUM")
            psB0 = tc.alloc_tile_pool(name=f"psB{r0}", bufs=2, space="PSUM")
            psC0 = tc.alloc_tile_pool(name=f"psC{r0}", bufs=2, space="PSUM")
            for tcn in range(4):
                blk_A(0, tcn, psA0)
            pri = [(lambda t=t: blk_A(0, t, psA0)) for t in range(4, TC_N)]
            pri += [(lambda t=t: blk_C(0, t, psC0)) for t in range(4)]
            sec = [(lambda fc=fc, nh=nh: blk_B(0, fc, nh, psB0))
                   for fc, nh in b_pair_order if nh == 0]
            interleave(pri, sec)
            pri = [(lambda t=t: blk_C(0, t, psC0)) for t in range(4, TC_N)]
            sec = [(lambda fc=fc, nh=nh: blk_B(0, fc, nh, psB0))
                   for fc, nh in b_pair_order if nh == 1]
            interleave(pri, sec)
            psC0.release()
            psB0.release()
            psA0.release()

            # -- stretch 1: D0 (ACT-bound) over A1 B1 --
            psS0 = tc.alloc_tile_pool(name=f"psS{r0}", bufs=2, space="PSUM")
            psX0 = tc.alloc_tile_pool(name=f"psX{r0}", bufs=1, space="PSUM")
            pPT0 = tc.alloc_tile_pool(name=f"pPT{r0}", bufs=2)
            psB1 = tc.alloc_tile_pool(name=f"psB{r1}", bufs=1, space="PSUM")
            sec = [load_big_weights]
            sec += [(lambda t=t: blk_A(1, t)) for t in range(TC_N)]
            sec += [(lambda fc=fc, nh=nh: blk_B(1, fc, nh, psB1))
                    for fc, nh in b_pair_order]
            pri = [(lambda nh=nh, h=h: blk_D(0, nh, h, psS0, psX0, pPT0))
                   for nh in range(2) for h in range(H)]
            interleave(pri, sec)
            psB1.release()
            psX0.release()
            psS0.release()
            pPT0.release()

            # -- stretch 2 psum reserved first so D1 scores/exp overlap C1 --
            psS1 = tc.alloc_tile_pool(name=f"psS{r1}", bufs=2, space="PSUM")
            psX1 = tc.alloc_tile_pool(name=f"psX{r1}", bufs=1, space="PSUM")
            pPT1 = tc.alloc_tile_pool(name=f"pPT{r1}", bufs=2)

            # -- gap: C1 (V projection for batch 1) --
            psC1 = tc.alloc_tile_pool(name=f"psC{r1}", bufs=1, space="PSUM")
            for tcn in range(TC_N):
                blk_C(1, tcn, psC1)
            psC1.release()
            efg_pools = {}

            def efg_open(key, bufs=1):
                efg_pools[key] = tc.alloc_tile_pool(
                    name=f"ps{key}{r0}", bufs=bufs, space="PSUM")

            sec = [lambda: efg_open("E")]
            sec += [(lambda t=t: blk_E(0, t, efg_pools["E"])) for t in range(TC_N)]
            sec += [lambda: efg_pools["E"].release(), lambda: efg_open("F")]

            def f_burst(b, mc0):
                for mc in range(mc0, mc0 + 8):
                    blk_F(b, mc, efg_pools["F"])

            sec += [(lambda m=m: f_burst(0, m)) for m in range(0, MC_H, 8)]
            sec += [lambda: efg_pools["F"].release(), lambda: efg_open("G")]
            sec += [(lambda t=t: blk_G(0, t, efg_pools["G"])) for t in range(TC_N)]
            sec += [lambda: efg_pools["G"].release(), lambda: efg_open("E1", 1)]
            sec += [(lambda t=t: blk_E(1, t, efg_pools["E1"])) for t in range(4)]
            pri = [(lambda nh=nh, h=h: blk_D(1, nh, h, psS1, psX1, pPT1))
                   for nh in range(2) for h in range(H)]
            interleave(pri, sec)
            efg_pools["E1"].release()
            psX1.release()
            psS1.release()
            pPT1.release()

            # -- tail: E1 F1 G1 --
            psE1 = tc.alloc_tile_pool(name=f"psE{r1}b", bufs=3, space="PSUM")
            for tcn in range(4, TC_N):
                blk_E(1, tcn, psE1)
            psE1.release()
            psF1 = tc.alloc_tile_pool(name=f"psF{r1}", bufs=3, space="PSUM")
            for mc in range(MC_H):
                blk_F(1, mc, psF1)
            psF1.release()
            psG1 = tc.alloc_tile_pool(name=f"psG{r1}", bufs=3, space="PSUM")
            for ti in range(TC_N):
                blk_G(1, ti, psG1)
            psG1.release()

        const.release()
        misc.release()

    _split_sync_waits(nc)
    return nc


_NC_CACHE = {}


def _get_nc(reps=1, skip_gb2=False, skip_gb3=False):
    key = f"nc{reps}_{skip_gb2}_{skip_gb3}"
    if key not in _NC_CACHE:
        _NC_CACHE[key] = _build_nc(reps, skip_gb2, skip_gb3)
    return _NC_CACHE[key]


def kernel(x, ln1_g, ln1_b, qkv_w, qkv_b, proj_w, proj_b,
           ln2_g, ln2_b, fc1_w, fc1_b, fc2_w, fc2_b, ln3_g, ln3_b,
           **extra):
    x = np.ascontiguousarray(np.asarray(x, np.float32))
    f = lambda a: np.ascontiguousarray(np.asarray(a, np.float32))
    f8 = lambda a: np.ascontiguousarray(np.asarray(SW * a, E4NP))
    qkv_w, qkv_b = f(qkv_w), f(qkv_b)

    # Fold LN1 gamma/beta into QKV weights/bias (host, fp32).
    w_eff = np.asarray(ln1_g, np.float32)[:, None] * qkv_w
    b_eff = np.asarray(ln1_b, np.float32) @ qkv_w + qkv_b

    # V weights: 6 k-tile chunks of SW*w_v + a bias k-tile pair whose first
    # row is SW*b_v (multiplied on-device by the ones row in xnT k-tile 6).
    wv_pack = np.zeros((8 * 128, D), np.float32)
    wv_pack[:D] = SW * w_eff[:, 2 * D:]
    wv_pack[D] = SW * b_eff[2 * D:]

    common = {
        "w_qk": f8(w_eff[:, :2 * D]),
        "w_v": np.ascontiguousarray(np.asarray(wv_pack, E4NP)),
        "b_qk": f(b_eff[:2 * D]),
        "p_w": f8(proj_w), "pbr": f(proj_b),
        "w1": f8(fc1_w), "b1": f(fc1_b),
        "w2": f8(fc2_w), "b2": f(fc2_b),
        "g2": f(ln2_g), "bt2": f(ln2_b),
        "g3": f(ln3_g), "bt3": f(ln3_b),
    }
    in_maps = [dict(common, x=x[i * BLOC:(i + 1) * BLOC]) for i in range(NCORES)]

    skip_gb2 = bool(np.all(common["g2"] == 1.0) and np.all(common["bt2"] == 0.0))
    skip_gb3 = bool(np.all(common["g3"] == 1.0) and np.all(common["bt3"] == 0.0))
    nc = _get_nc(1, skip_gb2, skip_gb3)
    res = run_bass_kernel_spmd(nc, in_maps, core_ids=list(range(NCORES)))
    _NC_CACHE["last_result"] = res
    return np.concatenate([r["y"] for r in res.results], axis=0)
